# revision 22
# baseline (speedup 1.0000x reference)
"""Binary 3x3 conv: PE-mode transposes + fp8 DoubleRow matmuls.

out = alpha * (sign(x) conv sign(w)), NHWC, SAME padding.
Data-parallel over batch: each of the 8 cores handles 8 images.

Design: keep the SDMA fabric at the pure HBM floor (4 MiB read + 4 MiB
write per image) by doing BOTH layout transposes on the TensorEngine in
transpose-mode, and halving the conv's PE time with fp8e4 DoubleRow
matmuls (virtual K=256; the k-tile pair strides 128 B through the
channel-major tile, expressed with an overlapping raw AP).

Per-core pipeline per image:
  1. SWDGE cast-DMA (fp32 HBM -> bf16 SBUF) in 2 halves, row-major X
     [r=128, (pp,ci)] with 1 zeroed pad pixel each side.  No separate
     sign pass: the sign is folded into the transpose evict (step 2).
  2. PE transpose-mode, 65 tiles [128,128]: X tile c (pixels pp=2c,2c+1)
     -> PSUM bf16 [q=(wlo,ci), r], 8 tiles per bank (3 banks); evict =
     DVE tensor_scalar (is_ge 0, subtract 0.5) -> XT8 fp8 in {-.5,+.5}
     (alpha is pre-doubled to compensate); the two pad-pixel regions of
     XT8 are re-zeroed with two small memsets (pads must contribute 0).
     Chunks are emitted as closures interleaved between the PREVIOUS
     image's matmul groups (transpose-mode doesn't count as PE-busy for
     HAM; a 65-tile stretch would re-throttle the PE clock to 1.2 GHz).
  3. fp8 DoubleRow conv: per 8-row x 64-pair output block, 3 matmuls
     (kh in 1,0,2), lhsT=[128,(t=2),128] w8[kh], rhs=[128,(t=2),(r<=8),
     (c'=64)] -> psum fp32 [m=(par,co), n=r*64+c'].  The (r outer, c'
     inner) stream order makes the PSUM writes fully contiguous --
     with r innermost (stride 64) the same matmul measured 429 ns
     instead of ~270 (PSUM has 8 B cachelines; non-contiguous hurts).
  4. Scale-evict psum * (2*alpha)[m%64] ([128,1] fp32 AP) -> fp16 O4 on
     ACT (activation Copy, scale=) -- ACT has headroom once the
     standalone sign pass is gone (alpha cannot live in the fp8
     weights: 3 mantissa bits).
  5. PE transpose-mode 16 tiles/grp: O4 -> PSUM fp16 (2 banks); evicts
     split ACT/DVE -> T2 fp16 pixel-major (the psum n-layout makes the
     transposed partition map to a uniform HBM stride, keeping the
     store AP 3-dim).
  6. One SWDGE cast-DMA store per image fp16 -> fp32 HBM (4 MiB, 512 B
     HBM runs).
"""

import os
import sys

sys.path.insert(0, "/opt/trn_rl_repo")

import numpy as np
import ml_dtypes

import concourse.bass as bass
import concourse.mybir as mybir
from concourse.bass_types import AP
from concourse import masks
from concourse.tile import TileContext
from concourse.bass_utils import run_bass_kernel_spmd

N_CORES = 8
IMGS_PER_CORE = 8
H = W = 128
C = 64
ROW = W * C           # 8192 bf16 per image row (data)
XROW = ROW + 128      # 8320 = 65*128: 1 pad pixel (64ch) each side
NT = 65               # transpose tiles per image


def _split_multi_waits(nc):
    """Single-sync-wait walrus workaround (see baseline)."""
    n_new = 0
    for f in nc.m.functions:
        for bb in f.blocks:
            insts = bb.instructions
            if not any(
                i.sync_info is not None and len(i.sync_info.on_wait) > 1
                for i in insts
            ):
                continue
            new = []
            for inst in insts:
                si = inst.sync_info
                if si is not None and len(si.on_wait) > 1:
                    waits = list(si.on_wait)
                    for j, w in enumerate(waits[:-1]):
                        n_new += 1
                        new.append(mybir.InstNoOp(
                            name=f"{inst.name}-sw{j}",
                            engine=inst.engine,
                            bass_nofuse=True,
                            sync_info=mybir.SyncInfo(on_wait=[w], on_update=[]),
                        ))
                    si.on_wait.clear()
                    si.on_wait.append(waits[-1])
                new.append(inst)
            bb.instructions = new
    return n_new


def _pack_weights(w_fp: np.ndarray):
    """Host prep: fp8 sign weights for DoubleRow + fp32 alpha column.

    w8[kh][64*wlo+ci, t, m]: m<64 -> kw=2t+wlo (if <=2); m>=64 -> kw=
    2t+wlo-1 (if >=0); else 0.  alpha_col[m] = 2*alpha[m % 64] (the
    activations are encoded as +-0.5, so alpha is pre-doubled)."""
    alpha = np.mean(np.abs(w_fp), axis=(0, 1, 2)).astype(np.float32)  # (co,)
    s = np.where(w_fp >= 0, 1.0, -1.0).astype(np.float32)  # (kh,kw,ci,co)
    w8 = np.zeros((3, 128, 2, 128), np.float32)
    for kh in range(3):
        for wlo in range(2):
            for t in range(2):
                kw_e = 2 * t + wlo        # even outputs (par=0, m<64)
                if kw_e <= 2:
                    w8[kh, 64 * wlo:64 * wlo + 64, t, 0:64] = s[kh, kw_e]
                kw_o = 2 * t + wlo - 1    # odd outputs (par=1, m>=64)
                if kw_o >= 0:
                    w8[kh, 64 * wlo:64 * wlo + 64, t, 64:128] = s[kh, kw_o]
    w8 = w8.astype(ml_dtypes.float8_e4m3)
    alpha_col = (2.0 * np.tile(alpha, 2))[:, None].astype(np.float32)  # (128,1)
    return np.ascontiguousarray(w8), np.ascontiguousarray(alpha_col)


_PROGRAM_CACHE = {}


def _build_program(repeats: int = 1, skip: tuple = ()):
    key = (repeats, tuple(sorted(skip)))
    if key in _PROGRAM_CACHE:
        return _PROGRAM_CACHE[key]
    skip = set(skip)

    f32 = mybir.dt.float32
    f16 = mybir.dt.float16
    bf16 = mybir.dt.bfloat16
    fp8 = mybir.dt.float8e4
    Copy = mybir.ActivationFunctionType.Copy
    DR = mybir.MatmulPerfMode.DoubleRow
    GE = mybir.AluOpType.is_ge
    SUB = mybir.AluOpType.subtract

    nc = bass.Bass()
    x_d = nc.dram_tensor("x", (IMGS_PER_CORE, H, W, C), f32, kind="ExternalInput")
    w8_d = nc.dram_tensor("w8", (3, 128, 2, 128), fp8, kind="ExternalInput")
    al_d = nc.dram_tensor("alpha_col", (128, 1), f32, kind="ExternalInput")
    out_d = nc.dram_tensor("out", (IMGS_PER_CORE, H, W, C), f32, kind="ExternalOutput")

    x_flat = x_d.rearrange("i h w c -> i (h w c)")      # [8, 1048576]
    out_flat = out_d.rearrange("i h w c -> i (h w c)")  # [8, 1048576]

    with TileContext(nc) as tc:
        with (
            tc.tile_pool(name="wpool", bufs=1) as wpool,
            tc.tile_pool(name="xpool", bufs=3) as xpool,
            tc.tile_pool(name="x8pool", bufs=3) as x8pool,
            tc.tile_pool(name="tps", bufs=3, space="PSUM") as tps,
            tc.tile_pool(name="cps", bufs=3, space="PSUM") as cps,
            tc.tile_pool(name="ops", bufs=2, space="PSUM") as ops,
            tc.tile_pool(name="opool", bufs=2) as opool,
            tc.tile_pool(name="tpool", bufs=2) as tpool,
        ):
            w8_sb = wpool.tile([128, 768], fp8)
            nc.sync.dma_start(
                out=w8_sb.rearrange("k (i t m) -> k i t m", i=3, t=2),
                in_=w8_d.rearrange("i k t m -> k i t m"))
            w8v = w8_sb.rearrange("k (i t m) -> k i t m", i=3, t=2)
            alpha_sb = wpool.tile([128, 1], f32)
            nc.sync.dma_start(out=alpha_sb[:], in_=al_d[:])
            identb = wpool.tile([128, 128], bf16)
            masks.make_identity(nc, identb[:])
            identh = wpool.tile([128, 128], f16)
            masks.make_identity(nc, identh[:])
            if skip:
                XCONST = wpool.tile([128, XROW], bf16)
                nc.vector.memset(XCONST[:, :], 0.0)
                X8CONST = wpool.tile([128, XROW], fp8)
                nc.vector.memset(X8CONST[:, :], 0.0)
                O4CONST = wpool.tile([128, 8192], f16)
                nc.vector.memset(O4CONST[:, :], 0.0)

            def emit_load(img, parts=2):
                """1. load (cast fp32 -> bf16) in `parts` pieces, memset
                pads.  The sign is applied during the transpose evict, so
                the raw bf16 values flow into the transposes as-is."""
                if "cast" in skip:
                    return XCONST
                X = xpool.tile([128, XROW], bf16, tag="X")
                nc.vector.memset(X[:, 0:64], 0.0)
                nc.vector.memset(X[:, 64 + ROW:], 0.0)
                step = ROW // parts
                src = x_flat[img].rearrange("(h i) -> h i", h=128)
                for p in range(parts):
                    nc.gpsimd.dma_start(
                        out=X[:, 64 + p * step:64 + (p + 1) * step],
                        in_=src[:, p * step:(p + 1) * step])
                return X

            def make_intr_chunks(X):
                """2. closures for bf16 PE-transpose chunks (8 tiles per
                PSUM bank) + sign-evicts (bf16 psum -> fp8 {-.5,+.5}) on
                DVE, with the two pad-pixel regions re-zeroed.

                Returned as closures so the caller can interleave them with
                the previous image's matmuls (transpose-mode doesn't count
                as PE-busy for HAM; a 65-tile stretch would re-throttle the
                clock to 1.2 GHz)."""
                if "intr" in skip:
                    return X8CONST, []
                XT8 = x8pool.tile([128, XROW], fp8, tag="XT8")

                def chunk(j):
                    def go():
                        c1 = min(8 * j + 8, NT)
                        n = (c1 - 8 * j) * 128
                        pst = tps.tile([128, 1024], bf16, tag="pst")
                        for i, c in enumerate(range(8 * j, c1)):
                            nc.tensor.transpose(
                                pst[:, 128 * i:128 * i + 128],
                                X[:, 128 * c:128 * c + 128], identb[:])
                        nc.vector.tensor_scalar(
                            XT8[:, 1024 * j:1024 * j + n], pst[:, 0:n],
                            0.0, 0.5, GE, SUB)
                        # pads must contribute 0 to the conv: re-zero the
                        # left pad (tile 0, wlo=0) and right pad (tile 64,
                        # wlo=1) that the sign-evict just turned into +0.5
                        if j == 0:
                            nc.vector.memset(XT8[0:64, 0:128], 0.0)
                        if j == 8:
                            nc.vector.memset(XT8[64:128, 8192:8320], 0.0)
                    return go

                return XT8, [chunk(j) for j in range(9)]

            def emit_conv(img, XT8, next_chunks, prev_otr, ping=False):
                """3-4. conv for img, interleaving the next image's
                transpose chunks AND the previous image's out-transpose
                groups between conv groups (both are HAM-invisible, so
                they must stay sandwiched between matmul bursts).
                Returns the out-transpose closures for this image."""
                xt8_ap = XT8[:]
                pstride = xt8_ap.ap[0][0]

                def dr_rhs(free_off, cnt):
                    # free dims (t, c', r) -- r innermost (stride 1) so the
                    # SBUF-side stream is contiguous; the psum layout below
                    # (n = c'*8 + r) keeps the PSUM writes contiguous too
                    return AP(xt8_ap.tensor, xt8_ap.offset + free_off,
                              [[pstride, 128], [128, 2], [128, 64], [1, cnt]])

                if "mm" not in skip:
                    O4 = opool.tile([128, 8192], f16, tag="O4")
                else:
                    O4 = O4CONST
                O4v = O4.rearrange("m (c r) -> m c r", r=128)
                for grp in range(4):
                    # interleave 2-3 next-image transpose chunks per grp
                    lo = (9 * grp) // 4
                    hi = (9 * (grp + 1)) // 4
                    for j in range(lo, min(hi, len(next_chunks))):
                        next_chunks[j]()
                    # interleave 4 prev-image out-transpose groups per grp
                    # (front-loaded so the prev store issues after grp 1,
                    # keeping the out-DMA stream fed in the endgame)
                    for j in range(4 * grp, min(4 * grp + 4, len(prev_otr))):
                        prev_otr[j]()
                    if "mm" not in skip:
                        for blk in range(4):
                            h0 = grp * 32 + blk * 8
                            psum = cps.tile([128, 512], f32, tag="ps")
                            if ping:
                                # pipeline-fill HAM ping: runs as soon as
                                # the bank frees (while the real matmuls
                                # still wait on XT8); the kh=1 start=True
                                # matmul clears has_written, so this write
                                # is dead -- no new dependencies created
                                nc.tensor.matmul(
                                    psum[:, 0:128], identb[:], identb[:],
                                    start=True, stop=True)
                            # psum memory layout n = c'*8 + r: the matmul
                            # streams (c' outer, r inner), so writes land
                            # fully contiguous (non-contiguous PSUM writes
                            # cost ~+180 ns per matmul, 8 B cachelines)
                            psv = psum.rearrange("m (c r) -> m c r", r=8)
                            for idx, kh in enumerate((1, 0, 2)):
                                rbase = h0 + kh - 1
                                r_lo = max(0, -rbase)
                                r_hi = min(8, 128 - rbase)
                                rhs = dr_rhs(rbase + r_lo, r_hi - r_lo)
                                nc.tensor.matmul(
                                    psv[:, :, r_lo:r_hi], w8v[:, kh], rhs,
                                    start=(idx == 0), stop=(idx == 2),
                                    perf_mode=DR)
                            # evict psum (contiguous read) into the
                            # transpose-ready O4 layout f = c'*128 + r
                            # (strided 16 B-run write; ACT is 1x anyway)
                            nc.scalar.activation(
                                out=O4v[:, :, h0:h0 + 8],
                                in_=psum.rearrange("m (c r) -> m c r", r=8),
                                func=Copy, scale=alpha_sb[:, 0:1])

                def make_otr():
                    """5-6. out-transpose closures: per group, 8 tiles
                    [128,128] (one per w-pair c') -> PSUM partitions become
                    image ROWS; T2[h, w*64+co] is pixel-major so the store
                    is one 4 MiB DMA with 32 KiB-contiguous HBM runs per
                    partition (512 B runs measured only ~287 GB/s vs ~356
                    for contiguous)."""
                    if "otr" in skip:
                        return []
                    T2 = tpool.tile([128, 8192], f16, tag="T2")

                    def ogrp(g):
                        def go():
                            pso = ops.tile([128, 1024], f16, tag="pso")
                            for i in range(8):
                                cp = 8 * g + i
                                nc.tensor.transpose(
                                    pso[:, 128 * i:128 * i + 128],
                                    O4[:, 128 * cp:128 * cp + 128],
                                    identh[:])
                            dst0 = 1024 * g
                            nc.scalar.activation(
                                out=T2[:, dst0:dst0 + 512],
                                in_=pso[:, 0:512], func=Copy)
                            nc.vector.tensor_copy(
                                T2[:, dst0 + 512:dst0 + 1024],
                                pso[:, 512:1024])
                            if g == 7 and "store" not in skip:
                                nc.gpsimd.dma_start(
                                    out=out_flat[img].rearrange(
                                        "(h f) -> h f", h=128),
                                    in_=T2[:],
                                )
                        return go

                    return [ogrp(g) for g in range(8)]

                return make_otr()

            # --- HAM warmup: ~40 cold matmuls (~4.3 us) while image 0's
            # DMA streams, so the PE clock is at 2.4 GHz by the time real
            # work arrives (HAM needs ~3.4 us of matmul activity to
            # un-throttle from the 1.2 GHz idle default; transpose-mode
            # does not count) ---
            if "mm" not in skip:
                warm_ps = cps.tile([128, 512], f32, tag="ps")
                for _ in range(40):
                    nc.tensor.matmul(warm_ps[:, 0:128], identb[:], identb[:],
                                     start=True, stop=True)

            # --- software pipeline: img N's conv interleaves img N+1's
            # transpose chunks (keeps matmuls flowing through the PE);
            # loads lead by 2 images so the SWDGE queue and the X slots
            # never gate the transpose chain ---
            total = IMGS_PER_CORE * repeats
            Xs = {0: emit_load(0, parts=4)}
            if total > 1:
                Xs[1] = emit_load(1 % IMGS_PER_CORE)
            XT8_cur, chunks = make_intr_chunks(Xs.pop(0))
            for ch in chunks:
                ch()
                if "mm" not in skip:
                    # keep HAM's activity window fed during the priming
                    # phase (transpose-mode doesn't count as PE-busy)
                    for _ in range(2):
                        nc.tensor.matmul(warm_ps[:, 0:128], identb[:],
                                         identb[:], start=True, stop=True)
            otr = []
            for img_rep in range(total):
                if img_rep + 2 < total:
                    Xs[img_rep + 2] = emit_load((img_rep + 2) % IMGS_PER_CORE)
                if img_rep + 1 < total:
                    XT8_next, nchunks = make_intr_chunks(Xs.pop(img_rep + 1))
                else:
                    XT8_next, nchunks = [], []
                otr = emit_conv(img_rep % IMGS_PER_CORE, XT8_cur, nchunks,
                                otr, ping=(img_rep < 2))
                XT8_cur = XT8_next
            for go in otr:
                go()

    _split_multi_waits(nc)
    _PROGRAM_CACHE[key] = nc
    return nc


def _in_maps(x: np.ndarray, w8: np.ndarray, alpha_col: np.ndarray):
    maps = []
    for i in range(N_CORES):
        maps.append({
            "x": x[i * IMGS_PER_CORE:(i + 1) * IMGS_PER_CORE],
            "w8": w8,
            "alpha_col": alpha_col,
        })
    return maps


def _pack_inputs(x, w_fp):
    x = np.ascontiguousarray(x, dtype=np.float32)
    w8, alpha_col = _pack_weights(np.asarray(w_fp, dtype=np.float32))
    return _in_maps(x, w8, alpha_col)


def kernel(x: np.ndarray, w_fp: np.ndarray) -> np.ndarray:
    assert x.shape == (64, 128, 128, 64) and w_fp.shape == (3, 3, 64, 64)
    nc = _build_program()
    res = run_bass_kernel_spmd(nc, _pack_inputs(x, w_fp),
                               core_ids=list(range(N_CORES)))
    out = np.concatenate([r["out"] for r in res.results], axis=0)
    kernel.last_results = res
    return out
